# revision 54
# baseline (speedup 1.0000x reference)
"""Multi-head attention (B=2, S=2048, D=1024, H=16) on 8 trn2 NeuronCores.

Sharding: core c handles batch b=c//4 and query rows [512*(c%4), +512).

Key compaction: the mask is per-key (~50% zeros). Masked keys are dropped
host-side (exactly equivalent: exp(-1e9) == 0 in fp32) and the kept keys are
padded to SK=1536 (Binomial(2048,1/2) exceeds 1536 with prob ~1e-113); pad
keys get mask bias -1e5 so exp underflows to exactly 0.

K/V projection is sharded across the 4 cores of each batch group: core c
projects its own 384-key chunk L=c%4, exchanged with 4-core AllGathers.
Attention processes chunks in ring order [L, L+1, L+2, L+3 (mod 4)]: the
local chunk runs from SBUF while the AllGathers fly; remote chunks are read
from the AllGather output with runtime-offset DMAs ((pid+i)%4 * slot), and
the mask is host-rotated per core to match the processing order.

Device layouts are feature-major so no on-chip transposes are needed:
  - inputs passed as query.T/key.T/value.T [D, rows], weights as W.T [in,out]
  - scores computed transposed [k, q]; softmax across k (partitions):
      exp on ACT with the mask folded into the per-partition exp bias,
      numerator+denominator via a ones-column appended to V in the attn@V
      matmul (ones are memset locally, not communicated),
      denominators accumulated on 16 partitions -> one reciprocal_approx_fast
      per head pair + K=1 broadcast matmuls + one DVE multiply per pair
  - 1/sqrt(dk) folded into wq host-side; bv/bo folded into bo' = bo + wo@bv
"""

import sys

for _p in ("/opt/trn_rl_repo", "/root/.axon_site/_ro/trn_rl_repo"):
    if _p not in sys.path:
        sys.path.insert(0, _p)

import numpy as np
import ml_dtypes

B, S, D, H, DK = 2, 2048, 1024, 16, 64
NCORES = 8
MQ = 512           # query rows per core
P = 128            # partitions
NIT = D // P       # 8 input-feature tiles
NOT_ = D // P      # 8 output-feature tiles
VW = DK + 1        # 65: head dim + ones column

BF16 = ml_dtypes.bfloat16

_CACHE = {}


def _build(sk, taps=False):
    from concourse import bacc
    import concourse.mybir as mybir
    import concourse.tile as tile
    import concourse.bass as bass

    chk = sk // 4          # keys per core chunk
    ckt = chk // P         # key tiles per chunk
    nkt = sk // P          # key tiles total
    KSLOT = D * chk        # flat elements of one K.T chunk in agk_out
    VSLOT = chk * H * DK   # flat elements of one V chunk in agv_out

    nc = bacc.Bacc("TRN2", target_bir_lowering=False, debug=False)
    dt = mybir.dt

    qT = nc.dram_tensor("qT", [D, MQ], dt.bfloat16, kind="ExternalInput")
    kT = nc.dram_tensor("kT", [D, chk], dt.bfloat16, kind="ExternalInput")
    vT = nc.dram_tensor("vT", [D, chk], dt.bfloat16, kind="ExternalInput")
    wq = nc.dram_tensor("wq", [D, D], dt.bfloat16, kind="ExternalInput")
    wk = nc.dram_tensor("wk", [D, D], dt.bfloat16, kind="ExternalInput")
    wv = nc.dram_tensor("wv", [D, D], dt.bfloat16, kind="ExternalInput")
    wo = nc.dram_tensor("wo", [D, D], dt.bfloat16, kind="ExternalInput")
    bq = nc.dram_tensor("bq", [P, NOT_], dt.float32, kind="ExternalInput")
    bk = nc.dram_tensor("bk", [P, NOT_], dt.float32, kind="ExternalInput")
    maskb = nc.dram_tensor("maskb", [P, nkt], dt.float32, kind="ExternalInput")
    bob = nc.dram_tensor("bob", [1, D], dt.float32, kind="ExternalInput")
    out = nc.dram_tensor("out", [MQ, D], dt.float32, kind="ExternalOutput")

    agk_in = nc.dram_tensor("agk_in", [KSLOT], dt.bfloat16)
    agk_out = nc.dram_tensor("agk_out", [4 * KSLOT], dt.bfloat16)
    agv_in = nc.dram_tensor("agv_in", [VSLOT], dt.bfloat16)
    agv_out = nc.dram_tensor("agv_out", [4 * VSLOT], dt.bfloat16)

    tap_t = {}
    if taps:
        for i in (1, 2, 3):
            tap_t[f"k{i}"] = nc.dram_tensor(
                f"dbg_k{i}", [P, NOT_ * chk], dt.bfloat16,
                kind="ExternalOutput",
            )
            tap_t[f"v{i}"] = nc.dram_tensor(
                f"dbg_v{i}", [P, ckt * H * VW], dt.bfloat16,
                kind="ExternalOutput",
            )
        tap_t["av"] = nc.dram_tensor(
            "dbg_av", [P, NOT_ * MQ], dt.float32, kind="ExternalOutput"
        )
        tap_t["den"] = nc.dram_tensor(
            "dbg_den", [P, NOT_ * MQ], dt.float32, kind="ExternalOutput"
        )
        tap_t["q"] = nc.dram_tensor(
            "dbg_q", [P, NOT_ * MQ], dt.bfloat16, kind="ExternalOutput"
        )
        tap_t["ctx"] = nc.dram_tensor(
            "dbg_ctx", [P, NOT_ * MQ], dt.bfloat16, kind="ExternalOutput"
        )

    with tile.TileContext(nc) as tc:
        with (
            tc.tile_pool(name="w", bufs=2) as wpool,
            tc.tile_pool(name="stat", bufs=1) as stat,
            tc.tile_pool(name="inT", bufs=2) as inpool,
            tc.tile_pool(name="big", bufs=1) as big,
            tc.tile_pool(name="kc", bufs=2) as kcpool,
            tc.tile_pool(name="vc", bufs=2) as vcpool,
            tc.tile_pool(name="pT", bufs=10) as ppool,
            tc.tile_pool(name="sm", bufs=2) as sm,
            tc.tile_pool(name="outp", bufs=2) as outp,
            tc.tile_pool(name="psA", bufs=4, space="PSUM") as psA,
            tc.tile_pool(name="psB", bufs=2, space="PSUM") as psB,
        ):
            pid = nc.partition_id()

            # ---- persistent tiles ----
            QT_sb = big.tile([P, NOT_, MQ], dt.bfloat16, tag="QT")
            ctx_sb = big.tile([P, NOT_, MQ], dt.bfloat16, tag="ctx")
            # attn@V accumulators: head pair j packed on partitions
            # [hh*64:(hh+1)*64, j]; denominators on partitions [2j+hh] of den
            av_acc = big.tile([P, NOT_, MQ], dt.float32, tag="avacc")
            # head h's denominator at partition 32*(h%2), column group h//2
            # (partition starts must be 32-aligned; pair j shares column j so
            # one batched reciprocal covers both heads)
            den_acc = big.tile([P, NOT_, MQ], dt.float32, tag="denacc")
            bq_sb = stat.tile([P, NOT_], dt.float32, tag="bq")
            bk_sb = stat.tile([P, NOT_], dt.float32, tag="bk")
            mb_sb = stat.tile([P, nkt], dt.float32, tag="mb")
            bob_sb = stat.tile([P, D], dt.float32, tag="bob")
            ones_sb = stat.tile([P, DK], dt.bfloat16, tag="ones")

            nc.vector.memset(den_acc, 1.0)
            nc.sync.dma_start(out=bq_sb, in_=bq[:, :])
            nc.sync.dma_start(out=bk_sb, in_=bk[:, :])
            nc.sync.dma_start(out=mb_sb, in_=maskb[:, :])
            bob_bcast = bass.AP(
                tensor=bob.ap().tensor, offset=0, ap=[[0, P], [1, D]]
            )
            nc.sync.dma_start(out=bob_sb, in_=bob_bcast)
            nc.vector.memset(ones_sb, 1.0)

            def load_w(name, dram, split=1, eng=None):
                t = wpool.tile([P, NIT, D], dt.bfloat16, tag="w", name=name)
                src = dram.ap().rearrange("(t p) o -> p t o", p=P)
                step = NIT // split
                for s in range(split):
                    sl = slice(s * step, (s + 1) * step)
                    (eng or nc.sync).dma_start(
                        out=t[:, sl, :], in_=src[:, sl, :]
                    )
                return t

            def proj_group(ps, w_sb, x_sb, m_slice, n_slice, swap=False):
                for it in range(NIT):
                    lhsT = (
                        x_sb[:, it, m_slice] if swap else w_sb[:, it, m_slice]
                    )
                    rhs = x_sb[:, it, n_slice] if not swap else w_sb[:, it, n_slice]
                    nc.tensor.matmul(
                        ps,
                        lhsT=lhsT,
                        rhs=rhs,
                        start=(it == 0),
                        stop=(it == NIT - 1),
                    )

            # ---- local K chunk projection + AllGather launch ----
            wk_sb = load_w("wk_sb", wk, split=4)
            kTl = inpool.tile([P, NIT, chk], dt.bfloat16, tag="inT")
            ksrc = kT.ap().rearrange("(t p) k -> p t k", p=P)
            for s in range(2):
                nc.sync.dma_start(
                    out=kTl[:, 4 * s : 4 * s + 4, :],
                    in_=ksrc[:, 4 * s : 4 * s + 4, :],
                )
            KTl = kcpool.tile([P, NOT_, chk], dt.bfloat16, tag="KTc")
            for ot in range(NOT_):
                ps = psA.tile([P, MQ], dt.float32, tag="ps1")
                proj_group(
                    ps[:, 0:chk], wk_sb, kTl, slice(ot * P, (ot + 1) * P),
                    slice(None),
                )
                nc.vector.tensor_scalar_add(
                    out=KTl[:, ot, :], in0=ps[:, 0:chk],
                    scalar1=bk_sb[:, ot : ot + 1],
                )
            nc.sync.dma_start(
                out=agk_in.ap().rearrange("(t p k) -> p t k", p=P, k=chk),
                in_=KTl,
            )
            nc.gpsimd.collective_compute(
                "AllGather",
                mybir.AluOpType.bypass,
                ins=[agk_in[:]],
                outs=[agk_out[:]],
                replica_groups=[[0, 1, 2, 3], [4, 5, 6, 7]],
            )

            # ---- local V chunk projection + AllGather launch ----
            # Vpl holds [keys(P), rt, head-major V + ones col]; only the pure
            # V columns ride the AllGather, ones are memset locally.
            # V/Q inputs ride the Activation hwdge queue (idle this early) so
            # they don't serialize behind the K path + staging on SP
            wv_sb = load_w("wv_sb", wv, split=4, eng=nc.scalar)
            vTl = inpool.tile([P, NIT, chk], dt.bfloat16, tag="inT")
            vsrc0 = vT.ap().rearrange("(t p) k -> p t k", p=P)
            for s in range(2):
                nc.scalar.dma_start(
                    out=vTl[:, 4 * s : 4 * s + 4, :],
                    in_=vsrc0[:, 4 * s : 4 * s + 4, :],
                )
            Vpl = vcpool.tile([P, ckt, H * VW], dt.bfloat16, tag="Vpc")
            vones = Vpl.rearrange("p t (h x) -> p t h x", x=VW)[
                :, :, :, DK : DK + 1
            ]
            nc.vector.memset(vones, 1.0)
            for rt in range(ckt):
                for oc in range(2):
                    ps = psA.tile([P, MQ], dt.float32, tag="ps1")
                    proj_group(
                        ps,
                        wv_sb,
                        vTl,
                        slice(rt * P, (rt + 1) * P),
                        slice(oc * 512, (oc + 1) * 512),
                        swap=True,
                    )
                    dst = Vpl[
                        :, rt, oc * 8 * VW : (oc * 8 + 8) * VW
                    ].rearrange("p (h x) -> p h x", x=VW)[:, :, 0:DK]
                    nc.vector.tensor_copy(
                        out=dst, in_=ps.rearrange("p (h x) -> p h x", x=DK)
                    )
            for rt in range(ckt):
                vsrc = Vpl.rearrange("p t (h x) -> p t h x", x=VW)[
                    :, rt, :, 0:DK
                ]
                dst = bass.AP(
                    tensor=agv_in.ap().tensor,
                    offset=rt * (P * H * DK),
                    ap=[[H * DK, P], [DK, H], [1, DK]],
                )
                nc.sync.dma_start(out=dst, in_=vsrc)
            nc.gpsimd.collective_compute(
                "AllGather",
                mybir.AluOpType.bypass,
                ins=[agv_in[:]],
                outs=[agv_out[:]],
                replica_groups=[[0, 1, 2, 3], [4, 5, 6, 7]],
            )

            # ---- Q projection (runs while the AllGathers fly) ----
            wq_sb = load_w("wq_sb", wq, split=4, eng=nc.scalar)
            qT_sb = inpool.tile([P, NIT, MQ], dt.bfloat16, tag="inT")
            qsrc = qT.ap().rearrange("(t p) q -> p t q", p=P)
            for s in range(2):
                nc.scalar.dma_start(
                    out=qT_sb[:, 4 * s : 4 * s + 4, :],
                    in_=qsrc[:, 4 * s : 4 * s + 4, :],
                )
            for ot in range(NOT_):
                ps = psA.tile([P, MQ], dt.float32, tag="ps1")
                proj_group(
                    ps, wq_sb, qT_sb, slice(ot * P, (ot + 1) * P), slice(None)
                )
                nc.vector.tensor_scalar_add(
                    out=QT_sb[:, ot, :], in0=ps, scalar1=bq_sb[:, ot : ot + 1]
                )

            # ---- attention over 4 chunks in ring order ----
            # pass 0: local chunk from SBUF. passes 1-3: chunk (pid+i)%4 from
            # the AllGather outputs via runtime-offset DMA. maskb columns are
            # host-rotated to match this order.
            def fetch_k(i):
                slot = (pid + i) % 4
                t = kcpool.tile([P, NOT_, chk], dt.bfloat16, tag="KTc",
                                name=f"KTc{i}")
                src = bass.AP(
                    tensor=agk_out.ap().tensor,
                    offset=slot * KSLOT,
                    ap=[[chk, P], [P * chk, NOT_], [1, chk]],
                    dep_tracking_offset=0,
                )
                nc.sync.dma_start(out=t, in_=src)
                return t

            def fetch_v(i):
                slot = (pid + i) % 4
                t = vcpool.tile([P, ckt, H * VW], dt.bfloat16, tag="Vpc",
                                name=f"Vpc{i}")
                if i == 1:
                    # ones columns are memset only on each pool buffer's
                    # FIRST use (Vpl=buf0, Vpc1=buf1): the layout repeats, so
                    # later fetches inherit them. A memset concurrent with
                    # the V-payload DMA corrupts adjacent elements (shared
                    # SBUF words); Vpc1's memset has no deps and runs long
                    # before AGv can land, so it never overlaps its DMA.
                    tones = t.rearrange("p t (h x) -> p t h x", x=VW)[
                        :, :, :, DK : DK + 1
                    ]
                    nc.vector.memset(tones, 1.0)
                for rt in range(ckt):
                    src = bass.AP(
                        tensor=agv_out.ap().tensor,
                        offset=slot * VSLOT + rt * (P * H * DK),
                        ap=[[H * DK, P], [DK, H], [1, DK]],
                        dep_tracking_offset=rt * (P * H * DK),
                    )
                    dstv = t.rearrange("p t (h x) -> p t h x", x=VW)[
                        :, rt, :, 0:DK
                    ]
                    nc.sync.dma_start(out=dstv, in_=src)
                return t

            def emit_scores(i, KTc, j):
                # -> p tile [P, ckt, 1024] bf16 (heads 2j | 2j+1 per rt)
                p_t = ppool.tile([P, ckt, 1024], dt.bfloat16, tag="pT",
                                 name=f"p_{i}_{j}")
                for rt in range(ckt):
                    sc = psB.tile([P, 1024], dt.float32, tag="ps2")
                    nc.tensor.matmul(
                        sc[:, 0:512],
                        lhsT=KTc[0:DK, j, rt * P : (rt + 1) * P],
                        rhs=QT_sb[0:DK, j, :],
                        start=True,
                        stop=True,
                        tile_position=(0, 0),
                    )
                    nc.tensor.matmul(
                        sc[:, 512:1024],
                        lhsT=KTc[DK:P, j, rt * P : (rt + 1) * P],
                        rhs=QT_sb[DK:P, j, :],
                        start=True,
                        stop=True,
                        tile_position=(DK, 0),
                    )
                    nc.scalar.activation(
                        out=p_t[:, rt, :],
                        in_=sc,
                        func=mybir.ActivationFunctionType.Exp,
                        bias=mb_sb[:, ckt * i + rt : ckt * i + rt + 1],
                        scale=1.0,
                    )
                return p_t

            def emit_av(i, Vpc, j, p_t, merge_with=None):
                # merge_with: list of (i2, Vpc2, p2) continuing the same PSUM
                # accumulation, so several passes drain with a single DVE add
                avp = [
                    psA.tile([P, MQ], dt.float32, tag="ps1",
                             name=f"av_{i}_{j}_{m}")
                    for m in range(2)
                ]
                groups = [(i, Vpc, p_t)] + (merge_with or [])
                for gi, (gslot, gV, gp) in enumerate(groups):
                    for rt in range(ckt):
                        for hh in range(2):
                            nc.tensor.matmul(
                                avp[hh][0:VW, :],
                                lhsT=gV[
                                    :, rt,
                                    (2 * j + hh) * VW : (2 * j + hh + 1) * VW,
                                ],
                                rhs=gp[:, rt, hh * 512 : (hh + 1) * 512],
                                start=(gi == 0 and rt == 0),
                                stop=(
                                    gi == len(groups) - 1 and rt == ckt - 1
                                ),
                                skip_group_check=True,
                            )
                for hh in range(2):
                    dslice = den_acc[32 * hh : 32 * hh + 1, j, :]
                    if i == 0:
                        nc.vector.tensor_copy(
                            out=av_acc[hh * DK : (hh + 1) * DK, j, :],
                            in_=avp[hh][0:DK, :],
                        )
                        nc.vector.tensor_copy(
                            out=dslice, in_=avp[hh][DK : DK + 1, :]
                        )
                    else:
                        nc.vector.tensor_add(
                            out=av_acc[hh * DK : (hh + 1) * DK, j, :],
                            in0=av_acc[hh * DK : (hh + 1) * DK, j, :],
                            in1=avp[hh][0:DK, :],
                        )
                        nc.vector.tensor_add(
                            out=dslice, in0=dslice,
                            in1=avp[hh][DK : DK + 1, :],
                        )

            def emit_norm(j):
                # bf16 reciprocal of the two denominators of head pair j
                # (rows 0/32 of column j; other rows compute garbage nobody
                # reads), broadcast across partitions via K=1 bf16 matmuls,
                # then one fused full-width multiply
                stage = sm.tile([P, MQ], dt.bfloat16, tag="recip")
                with nc.allow_low_precision(
                    reason="softmax denominators only need ~3 digits"
                ):
                    nc.vector.reciprocal(
                        out=stage[0:DK, :], in_=den_acc[0:DK, j, :]
                    )
                ps_bc = psA.tile([P, MQ], dt.float32, tag="ps1")
                ps_bc2 = psA.tile([P, MQ], dt.float32, tag="ps1")
                nc.tensor.matmul(
                    ps_bc[0:DK, :],
                    lhsT=ones_sb[0:1, :],
                    rhs=stage[0:1, :],
                    start=True,
                    stop=True,
                )
                nc.tensor.matmul(
                    ps_bc2[0:DK, :],
                    lhsT=ones_sb[32:33, :],
                    rhs=stage[32:33, :],
                    start=True,
                    stop=True,
                )
                nc.vector.tensor_copy(
                    out=ps_bc[DK:P, :], in_=ps_bc2[0:DK, :]
                )
                nc.vector.tensor_mul(
                    out=ctx_sb[:, j, :],
                    in0=av_acc[:, j, :],
                    in1=ps_bc,
                )

            # pass 0 (local), fully from SBUF
            p_prev = None
            for j in range(NOT_):
                p_t = emit_scores(0, KTl, j)
                if p_prev is not None:
                    emit_av(0, Vpl, j - 1, p_prev)
                p_prev = p_t
            emit_av(0, Vpl, NOT_ - 1, p_prev)

            # remote passes. Fetch order matters twice over: K fetches are
            # emitted before V fetches so the AGv-gated V DMAs don't
            # head-of-line-block pass 2's K data on the DMA queue, and every
            # buffer-reusing fetch is emitted only after its buffer's readers
            # (WAR deps are computed at emission time). Scores of pass 2
            # (j0-1, capped by the p-tile pool) keep PE fed across the AGv
            # wait.
            # tile_wait_until marks AGv-gated work with a virtual not-before
            # time so the scheduler doesn't hoist it ahead of score work that
            # has data (the cost model doesn't know collective latency).
            # Values sit safely BELOW the observed AGv landing (~165-177us).
            KTc1 = fetch_k(1)
            KTc2 = fetch_k(2)   # reuses KTl's buf: local scores emitted
            with tc.tile_wait_until(0.140):
                Vpc1 = fetch_v(1)
                Vpc2 = fetch_v(2)   # reuses Vpl's buf: local AV emitted
            p1 = [emit_scores(1, KTc1, j) for j in range(NOT_)]
            KTc3 = fetch_k(3)   # reuses KTc1's buf: scores p1 emitted
            p2 = [emit_scores(2, KTc2, j) for j in range(2)]
            with tc.tile_wait_until(0.140):
                for j in range(NOT_):
                    emit_av(1, Vpc1, j, p1[j])
                Vpc3 = fetch_v(3)   # reuses Vpc1's buf: AV p1 emitted
            p2 += [emit_scores(2, KTc2, j) for j in range(2, NOT_)]
            # passes 2+3 share one PSUM accumulation per head pair -> one
            # DVE drain for both
            with tc.tile_wait_until(0.150):
                for j in range(NOT_):
                    p3 = emit_scores(3, KTc3, j)
                    emit_av(2, Vpc2, j, p2[j],
                            merge_with=[(3, Vpc3, p3)])
                    emit_norm(j)

            if taps:
                for i, (kt_, vt_) in enumerate(
                    [(KTc1, Vpc1), (KTc2, Vpc2), (KTc3, Vpc3)], start=1
                ):
                    nc.sync.dma_start(
                        out=tap_t[f"k{i}"].ap().rearrange(
                            "p (t k) -> p t k", k=chk
                        ),
                        in_=kt_,
                    )
                    nc.sync.dma_start(
                        out=tap_t[f"v{i}"].ap().rearrange(
                            "p (t c) -> p t c", c=H * VW
                        ),
                        in_=vt_,
                    )
                nc.sync.dma_start(
                    out=tap_t["av"].ap().rearrange("p (t q) -> p t q", q=MQ),
                    in_=av_acc,
                )
                nc.sync.dma_start(
                    out=tap_t["den"].ap().rearrange("p (t q) -> p t q", q=MQ),
                    in_=den_acc,
                )
                nc.sync.dma_start(
                    out=tap_t["q"].ap().rearrange("p (t q) -> p t q", q=MQ),
                    in_=QT_sb,
                )
                nc.sync.dma_start(
                    out=tap_t["ctx"].ap().rearrange("p (t q) -> p t q", q=MQ),
                    in_=ctx_sb,
                )

            # ---- output projection ----
            wo_sb = load_w("wo_sb", wo)
            for qt in range(MQ // P):
                for oc in range(2):
                    ps = psA.tile([P, MQ], dt.float32, tag="ps1")
                    for jt in range(NIT):
                        nc.tensor.matmul(
                            ps,
                            lhsT=ctx_sb[:, jt, qt * P : (qt + 1) * P],
                            rhs=wo_sb[:, jt, oc * 512 : (oc + 1) * 512],
                            start=(jt == 0),
                            stop=(jt == NIT - 1),
                        )
                    o_sb = outp.tile([P, MQ], dt.float32, tag="osb")
                    nc.vector.tensor_add(
                        out=o_sb,
                        in0=ps,
                        in1=bob_sb[:, oc * 512 : (oc + 1) * 512],
                    )
                    nc.sync.dma_start(
                        out=out[
                            qt * P : (qt + 1) * P, oc * 512 : (oc + 1) * 512
                        ],
                        in_=o_sb,
                    )

    nc.finalize()
    return nc


def _get_nc(sk):
    key = f"nc{sk}"
    if key not in _CACHE:
        _CACHE[key] = _build(sk)
    return _CACHE[key]


def _make_inputs(sk, query, key, value, mask, wq, bq, wk, bk, wv, bv, wo, bo):
    chk = sk // 4
    nkt = sk // P
    f32 = np.float32
    query = np.asarray(query, dtype=f32)
    key = np.asarray(key, dtype=f32)
    value = np.asarray(value, dtype=f32)
    mask = np.asarray(mask)
    wqT = np.ascontiguousarray(np.asarray(wq, f32).T / 8.0).astype(BF16)
    wkT = np.ascontiguousarray(np.asarray(wk, f32).T).astype(BF16)
    wvT = np.ascontiguousarray(np.asarray(wv, f32).T).astype(BF16)
    woT = np.ascontiguousarray(np.asarray(wo, f32).T).astype(BF16)
    bq8 = np.ascontiguousarray((np.asarray(bq, f32) / 8.0).reshape(NOT_, P).T)
    bkr = np.ascontiguousarray(np.asarray(bk, f32).reshape(NOT_, P).T)
    bob = (np.asarray(bo, f32) + np.asarray(wo, f32) @ np.asarray(bv, f32))[None, :]
    bob = np.ascontiguousarray(bob)
    onesr = np.ones((1, DK), dtype=f32)

    # compact keys per batch: keep unmasked, pad to sk with -1e5 mask bias
    kTc_b, vTc_b, mrows_b = [], [], []
    for b in range(B):
        idx = np.nonzero(mask[b, 0, 0] != 0)[0]
        kc = np.zeros((sk, D), dtype=f32)
        vc = np.zeros((sk, D), dtype=f32)
        kc[: len(idx)] = key[b][idx]
        vc[: len(idx)] = value[b][idx]
        mb = np.full(sk, -1e5, dtype=f32)
        mb[: len(idx)] = 0.0
        kTc_b.append(np.ascontiguousarray(kc.T).astype(BF16))
        vTc_b.append(np.ascontiguousarray(vc.T).astype(BF16))
        mrows_b.append(mb.reshape(nkt, P))

    in_maps = []
    for c in range(NCORES):
        b = c // 4
        L = c % 4
        q0 = L * MQ
        qTc = np.ascontiguousarray(query[b].T[:, q0 : q0 + MQ]).astype(BF16)
        # mask columns in ring processing order: pass i covers chunk (L+i)%4
        cols = []
        for i in range(4):
            ch = (L + i) % 4
            for t in range(chk // P):
                cols.append(mrows_b[b][ch * (chk // P) + t])
        mbias = np.ascontiguousarray(np.stack(cols, axis=1))
        in_maps.append(
            {
                "qT": qTc,
                "kT": np.ascontiguousarray(kTc_b[b][:, L * chk : (L + 1) * chk]),
                "vT": np.ascontiguousarray(vTc_b[b][:, L * chk : (L + 1) * chk]),
                "wq": wqT,
                "wk": wkT,
                "wv": wvT,
                "wo": woT,
                "bq": bq8,
                "bk": bkr,
                "maskb": mbias,
                "bob": bob,
                "onesr": onesr,
            }
        )
    return in_maps


def kernel(query, key, value, mask, wq, bq, wk, bk, wv, bv, wo, bo):
    from concourse.bass_utils import run_bass_kernel_spmd

    mask_np = np.asarray(mask)
    nkeep = max(
        int(np.count_nonzero(mask_np[b, 0, 0] != 0)) for b in range(B)
    )
    sk = 1536 if nkeep <= 1536 else 2048

    nc = _get_nc(sk)
    in_maps = _make_inputs(
        sk, query, key, value, mask, wq, bq, wk, bk, wv, bv, wo, bo
    )
    res = run_bass_kernel_spmd(nc, in_maps, core_ids=list(range(NCORES)))
    _CACHE["last_result"] = res
    out = np.empty((B, S, D), dtype=np.float32)
    for c in range(NCORES):
        b = c // 4
        q0 = (c % 4) * MQ
        out[b, q0 : q0 + MQ, :] = res.results[c]["out"]
    return out
